# revision 24
# baseline (speedup 1.0000x reference)
"""Trainium2 Bass kernel for nn_AttentionSubgraphExtractor (topk_masking).

Math notes (derived from the reference):
  - Q/Wq/bq cancel: softmax over the peptide axis removes any per-(b,m)
    constant, so attention depends only on sp[b,p] = H_p[b,p,:] @ (Wk.T @ a2)
    (bk shifts all p equally -> no effect on ordering).
  - softmax is monotonic -> top-k set of att == top-k set of sp.
  - mask is constant per row m: 0 for m in INDICES else -1e9.  att in (0,1)
    added to -1e9 is exactly -1e9 in f32, so masked rows are all-tied and
    jax.lax.top_k returns indices [0..19].
  - The scatter only cares about the SET of top-20 indices.  With s[p] =
    1 if p in top-20 of sp[b], t[p] = 1 if p < 20, u[m] = 1 if m in INDICES:
      A[b, m, 300+p]  = u[m]*s[p] + (1-u[m])*t[p]      (m < 300)
      A[b, 300+p, c]  = u[c]*s[p] + (1-u[c])*t[p]      (c < 300), 0 for c>=300
      A[b, :300,:300] = eye(300), A[b,300:,300:] = 0
  - H_sub = concat(H_m, H_p) -> pure DMA.

So the kernel is memory-bound: stream H_m/H_p through to H_sub, and write
A_sub from SBUF-resident templates patched per batch with the s-dependent
parts.  A_mm (184 MB) is never used and is not transferred.

Sharding: pure data parallelism, batch 512 -> 64 per core across 8 cores.
"""

import numpy as np

import concourse.bass as bass
import concourse.bacc as bacc
import concourse.mybir as mybir
from concourse.tile import TileContext
from concourse.vector_clock import ScopedClock
from concourse.bass_utils import run_bass_kernel_spmd

F32 = mybir.dt.float32

N_CORES = 8
B_TOTAL = 512
BL = B_TOTAL // N_CORES        # 64 batches per core
M = 300                        # mhc nodes
P = 24                         # peptide nodes
D = 512                        # hidden
SIZE = 324                     # 300 + 24
TOPK = 20
AD = 64                        # attention dim
GB = 8                         # batches per A_sub slab group
NG = BL // GB                  # groups per core
RING = 2                       # slab ring depth
HM_CHUNK = 8                   # batches per H_m dram->dram DMA

INDICES = [7, 9, 24, 45, 59, 62, 63, 66, 67, 69, 70, 73, 74, 76, 77, 80, 81,
           84, 95, 97, 99, 114, 116, 118, 143, 147, 150, 152, 156, 158, 159,
           163, 167, 171]


class _SplitDrainTileContext(TileContext):
    """TileContext whose tail drain carries at most one sem wait.

    The pinned walrus rejects a TPB_CTRL (Drain) instruction with more than
    one sync-wait ("Too many sync wait commands", CoreV3GenImpl.cpp:104), and
    the stock tail drain waits on every outstanding sem lane.  Split it into
    a chain of single-wait drains on SP; semantically identical.
    """

    def _drain_and_barrier(self, tick_clock, wait_clock):
        drain_inst = self.nc.sync.drain()
        wait_clock.add_sem_waits(
            drain_inst.ins, ScopedClock({None: tick_clock.global_clock})
        )
        si = drain_inst.ins.sync_info
        if si is not None and si.on_wait and len(si.on_wait) > 1:
            waits = list(si.on_wait)
            drain_inst.ins.sync_info = mybir.SyncInfo(
                on_wait=[waits[0]], on_update=list(si.on_update or [])
            )
            for w in waits[1:]:
                d2 = self.nc.sync.drain()
                d2.ins.sync_info = mybir.SyncInfo(on_wait=[w], on_update=[])
        self.nc.all_engine_barrier()
        assert self.sems is not None
        popped = self.nc._tile_sem_poison_stack.pop()
        assert popped is self._sem_poison
        self.nc.clear_and_free_semaphores(list(self.sems.allocated().values()))
        self.nc.all_engine_barrier()


def build_kernel() -> bass.Bass:
    nc = bacc.Bacc("TRN2")

    H_m = nc.dram_tensor("H_m", [BL, M, D], F32, kind="ExternalInput")
    H_p = nc.dram_tensor("H_p", [BL, P, D], F32, kind="ExternalInput")
    Wk = nc.dram_tensor("Wk", [AD, D], F32, kind="ExternalInput")
    av = nc.dram_tensor("att_vec", [2 * AD], F32, kind="ExternalInput")
    A_out = nc.dram_tensor("A_sub", [BL, SIZE, SIZE], F32, kind="ExternalOutput")
    H_out = nc.dram_tensor("H_sub", [BL, SIZE, D], F32, kind="ExternalOutput")

    with _SplitDrainTileContext(nc) as tc:
        with (
            tc.tile_pool(name="big", bufs=1) as big,
            tc.tile_pool(name="small", bufs=1) as small,
            tc.tile_pool(name="pw", bufs=1, space="PSUM") as pw,
            tc.tile_pool(name="pstrip", bufs=3, space="PSUM") as pstrip,
        ):
            # ---------- loads ----------
            hp_sb = big.tile([BL, P * D], F32, name="hp_sb")        # 48 KB/part
            nc.sync.dma_start(out=hp_sb[:], in_=H_p[:])

            wk_sb = small.tile([AD, D], F32, name="wk_sb")
            nc.sync.dma_start(out=wk_sb[:], in_=Wk[:])
            a2_col = small.tile([AD, 1], F32, name="a2_col")
            nc.sync.dma_start(out=a2_col[:], in_=av[AD:2 * AD])

            # ---------- constants ----------
            ones128 = small.tile([128, M], F32, name="ones128")
            nc.vector.memset(ones128[:], 1.0)
            t_row = small.tile([1, P], F32, name="t_row")
            nc.vector.memset(t_row[:, 0:TOPK], 1.0)
            nc.vector.memset(t_row[:, TOPK:P], 0.0)

            # u as a single row [1, 300]; strided views u[4q+j] are the
            # lhsT operands of the strip outer-product matmuls
            u4row = small.tile([1, M], F32, name="u4row")
            nc.vector.memset(u4row[:], 0.0)
            for idx in INDICES:
                nc.vector.memset(u4row[:, idx:idx + 1], 1.0)

            # U324/T324: bottom-row constants (24 partitions)
            U324 = small.tile([P, SIZE], F32, name="U324")
            nc.vector.memset(U324[:], 0.0)
            for idx in INDICES:
                nc.vector.memset(U324[:, idx:idx + 1], 1.0)
            T324 = small.tile([P, SIZE], F32, name="T324")
            nc.vector.memset(T324[:], 0.0)
            nc.vector.memset(T324[0:TOPK, 0:M], 1.0)

            # eye masters + identity (GpSimd affine_select is slow: build each
            # pattern ONCE, replicate with broadcast-read copies on ScalarE)
            I64 = small.tile([BL, BL], F32, name="I64")
            nc.gpsimd.affine_select(
                I64[:], ones128[0:BL, 0:BL], pattern=[[1, BL]],
                compare_op=mybir.AluOpType.is_equal, fill=0.0,
                base=0, channel_multiplier=-1,
            )
            # eye master in 4-rows-per-partition layout: partition q holds
            # rows 4q..4q+3; eye4[q, (r, c)] = 1 iff c == 4q + r
            QP = M // 4                                   # 75 partitions
            RW = 4 * SIZE                                 # 1296 elems/slot
            ones4 = small.tile([QP, RW], F32, name="ones4")
            nc.vector.memset(ones4[:], 1.0)
            eye4 = small.tile([QP, RW], F32, name="eye4")
            nc.gpsimd.affine_select(
                eye4[:].rearrange("q (r c) -> q r c", c=SIZE),
                ones4[:].rearrange("q (r c) -> q r c", c=SIZE),
                pattern=[[-1, 4], [1, SIZE]],
                compare_op=mybir.AluOpType.is_equal, fill=0.0,
                base=0, channel_multiplier=-4)

            # All bulk transfers go on the single SWDGE queue (q0), hand-
            # interleaved: H_m chunks first (independent), then alternating
            # with A_sub group writes as they are produced.  One FIFO =
            # engines never starve and no cross-queue priority effects.
            for c in range(4):
                g0 = c * HM_CHUNK
                nc.gpsimd.dma_start(out=H_out[g0:g0 + HM_CHUNK, 0:M, :],
                                    in_=H_m[g0:g0 + HM_CHUNK])

            # ---------- w = Wk.T @ a2; replications via PE outer products ---
            # (keeps POOL off the compute critical path entirely)
            w_ps = pw.tile([1, D], F32, tag="wps")
            nc.tensor.matmul(w_ps[:], lhsT=a2_col[:], rhs=wk_sb[:])
            w_row = small.tile([1, D], F32, name="w_row")
            nc.vector.tensor_copy(out=w_row[:], in_=w_ps[:])
            # w replicated to 64 partitions: ones(64) ⊗ w  (PSUM-resident)
            wrep_ps = pw.tile([BL, D], F32, tag="wrep")
            nc.tensor.matmul(wrep_ps[:], lhsT=ones128[0:1, 0:BL], rhs=w_row[:])
            # t replicated to 64 partitions (for d01) and 128 (strip template)
            trep_ps = pw.tile([BL, P], F32, tag="trep")
            nc.tensor.matmul(trep_ps[:], lhsT=ones128[0:1, 0:BL], rhs=t_row[:])
            # t tiled x4 then replicated to 75 partitions (strip template)
            t4_row = small.tile([1, 4 * P], F32, name="t4_row")
            nc.vector.tensor_copy(
                out=t4_row[:].rearrange("a (r p) -> a r p", p=P),
                in_=t_row[:].unsqueeze(1).to_broadcast([1, 4, P]))
            tstrip_ps = pw.tile([QP, 4 * P], F32, tag="tstrip")
            nc.tensor.matmul(tstrip_ps[:], lhsT=ones128[0:1, 0:QP], rhs=t4_row[:])
            T4_strip = small.tile([QP, 4 * P], F32, name="T4_strip")
            nc.vector.tensor_copy(out=T4_strip[:], in_=tstrip_ps[:])

            # ---------- A_sub slab templates (4 rows per partition) -------
            slabs4 = [big.tile([QP, GB * RW], F32, name=f"slab4_{r}")
                      for r in range(RING)]
            slabs2a = [big.tile([P, GB * SIZE], F32, name=f"slab2a_{r}")
                       for r in range(RING)]

            for r in range(RING):
                # replicate the eye master into all GB slots in one
                # broadcast-read copy per slab (on ScalarE; ACT is idle early)
                nc.scalar.copy(
                    out=slabs4[r][:].rearrange("q (g x) -> q g x", x=RW),
                    in_=eye4[:].unsqueeze(1).to_broadcast([QP, GB, RW]))

            # H_sub peptide rows back out of SBUF (ACT stream, before A DMAs)
            nc.scalar.dma_start(out=H_out[:, M:SIZE, :], in_=hp_sb[:])

            # ---------- sp[b,p] = sum_d H_p[b,p,d] * w[d] ----------
            sp = small.tile([BL, P], F32, name="sp")
            dummy = small.tile([BL, D], F32, name="dummy")
            for p in range(P):
                nc.vector.scalar_tensor_tensor(
                    out=dummy[:], in0=hp_sb[:, p * D:(p + 1) * D], scalar=1.0,
                    in1=wrep_ps[:], op0=mybir.AluOpType.mult,
                    op1=mybir.AluOpType.mult, accum_out=sp[:, p:p + 1],
                )

            # ---------- s01 = top-20 mask of sp per batch ----------
            cmp = small.tile([BL, P * P], F32, name="cmp")
            cmp3 = cmp[:].rearrange("b (p q) -> b p q", q=P)
            sp_q = sp[:].unsqueeze(1).to_broadcast([BL, P, P])   # (b,p,q)->sp[b,q]
            sp_p = sp[:].unsqueeze(2).to_broadcast([BL, P, P])   # (b,p,q)->sp[b,p]
            nc.vector.tensor_tensor(out=cmp3, in0=sp_q, in1=sp_p,
                                    op=mybir.AluOpType.is_lt)
            cnt = small.tile([BL, P], F32, name="cnt")
            nc.vector.tensor_reduce(out=cnt[:], in_=cmp3, axis=mybir.AxisListType.X,
                                    op=mybir.AluOpType.add)
            s01 = small.tile([BL, P], F32, name="s01")
            nc.vector.tensor_scalar(out=s01[:], in0=cnt[:], scalar1=float(P - TOPK) - 0.5,
                                    scalar2=None, op0=mybir.AluOpType.is_gt)
            # d01 = s01 - t  (t replicated via PE, PSUM-resident)
            d01 = small.tile([BL, P], F32, name="d01")
            nc.vector.tensor_sub(out=d01[:], in0=s01[:], in1=trep_ps[:])

            # d01T[p, b] = d01[b, p] via PE transpose
            dT_ps = pw.tile([P, BL], F32, tag="dTps")
            nc.tensor.matmul(dT_ps[:], lhsT=d01[:], rhs=I64[:])
            d01T = small.tile([P, BL], F32, name="d01T")
            nc.vector.tensor_copy(out=d01T[:], in_=dT_ps[:])

            # dRow_all[0, b*24+p] = d01[b, p]: partition->free flatten via a
            # small SBUF->SBUF DMA on ACT (compute-gated; ACT reaches it at
            # about the same time d01 lands, so nothing stalls)
            dRow_all = small.tile([1, BL * P], F32, name="dRow_all")
            nc.sync.dma_start(
                out=dRow_all[:].rearrange("a (g p) -> a g p", p=P),
                in_=d01[:],
            )

            # ---------- per-batch A_sub assembly + streaming ----------
            for b in range(BL):
                g, slot = divmod(b, GB)
                r = g % RING
                c0 = slot * SIZE
                d_row = dRow_all[:, b * P:(b + 1) * P]       # [1, 24]

                # strips for rows 0:300 in 4-row layout: block j gets
                # u[4q+j] ⊗ d_b (4 outer products into one PSUM bank), then
                # + t template in one DVE add straight into the slab
                sps = pstrip.tile([QP, 4 * P], F32, tag="ps")
                for j in range(4):
                    u_j = u4row[:].rearrange("a (q j) -> a q j", j=4)[:, :, j:j + 1]
                    nc.tensor.matmul(sps[:, j * P:(j + 1) * P],
                                     lhsT=u_j, rhs=d_row)
                strip_dst = slabs4[r][:].rearrange(
                    "q (g r c) -> q g r c", r=4, c=SIZE)[:, slot, :, M:SIZE]
                nc.vector.tensor_add(out=strip_dst, in0=sps[:], in1=T4_strip[:])

                # bottom rows: U324*d_b + T324, covers all 324 cols
                nc.vector.scalar_tensor_tensor(
                    out=slabs2a[r][:, c0:c0 + SIZE], in0=U324[:],
                    scalar=d01T[:, b:b + 1], in1=T324[:],
                    op0=mybir.AluOpType.mult, op1=mybir.AluOpType.add,
                )

                if slot == GB - 1:
                    g0 = g * GB
                    src4 = slabs4[r][:].rearrange("q (g x) -> q g x", x=RW)
                    dst4 = A_out[g0:g0 + GB, 0:M, :].rearrange(
                        "b (q r) c -> q b r c", r=4)
                    nc.gpsimd.dma_start(out=dst4, in_=src4)
                    src2a = slabs2a[r][:].rearrange("p (g c) -> p g c", c=SIZE)
                    nc.gpsimd.dma_start(
                        out=A_out[g0:g0 + GB, M:SIZE, :].transpose([1, 0, 2]),
                        in_=src2a)
                    if g < 4:
                        h0 = (4 + g) * HM_CHUNK
                        nc.gpsimd.dma_start(
                            out=H_out[h0:h0 + HM_CHUNK, 0:M, :],
                            in_=H_m[h0:h0 + HM_CHUNK])

            # H_p rows of H_sub, last on the bulk queue
            nc.gpsimd.dma_start(out=H_out[:, M:SIZE, :], in_=hp_sb[:])

    nc.compile()
    return nc


_NC_CACHE: list = []


def _get_nc() -> bass.Bass:
    if not _NC_CACHE:
        _NC_CACHE.append(build_kernel())
    return _NC_CACHE[0]


def run_sharded(inputs: dict, **spmd_kwargs):
    """Shard full inputs over 8 cores, run, and return (results, perf)."""
    H_m = np.ascontiguousarray(np.asarray(inputs["H_m"], dtype=np.float32))
    H_p = np.ascontiguousarray(np.asarray(inputs["H_p"], dtype=np.float32))
    Wk = np.ascontiguousarray(np.asarray(inputs["Wk"], dtype=np.float32))
    av = np.ascontiguousarray(np.asarray(inputs["att_vec"], dtype=np.float32))
    assert H_m.shape == (B_TOTAL, M, D) and H_p.shape == (B_TOTAL, P, D)

    nc = _get_nc()
    in_maps = []
    for c in range(N_CORES):
        sl = slice(c * BL, (c + 1) * BL)
        in_maps.append({
            "H_m": H_m[sl],
            "H_p": H_p[sl],
            "Wk": Wk,
            "att_vec": av,
        })
    res = run_bass_kernel_spmd(nc, in_maps, core_ids=list(range(N_CORES)),
                               **spmd_kwargs)
    A = np.concatenate([res.results[c]["A_sub"] for c in range(N_CORES)], axis=0)
    H = np.concatenate([res.results[c]["H_sub"] for c in range(N_CORES)], axis=0)
    return (A, H), res


def kernel(H_m, H_p, A_mm, Wq, bq, Wk, bk, att_vec):
    """Full-input entry point: returns (A_sub, H_sub) like the reference."""
    out, _ = run_sharded({"H_m": H_m, "H_p": H_p, "Wk": Wk, "att_vec": att_vec})
    return out


# revision 25
# speedup vs baseline: 1.0543x; 1.0543x over previous
"""Trainium2 Bass kernel for nn_AttentionSubgraphExtractor (topk_masking).

Math notes (derived from the reference):
  - Q/Wq/bq cancel: softmax over the peptide axis removes any per-(b,m)
    constant, so attention depends only on sp[b,p] = H_p[b,p,:] @ (Wk.T @ a2)
    (bk shifts all p equally -> no effect on ordering).
  - softmax is monotonic -> top-k set of att == top-k set of sp.
  - mask is constant per row m: 0 for m in INDICES else -1e9.  att in (0,1)
    added to -1e9 is exactly -1e9 in f32, so masked rows are all-tied and
    jax.lax.top_k returns indices [0..19].
  - The scatter only cares about the SET of top-20 indices.  With s[p] =
    1 if p in top-20 of sp[b], t[p] = 1 if p < 20, u[m] = 1 if m in INDICES:
      A[b, m, 300+p]  = u[m]*s[p] + (1-u[m])*t[p]      (m < 300)
      A[b, 300+p, c]  = u[c]*s[p] + (1-u[c])*t[p]      (c < 300), 0 for c>=300
      A[b, :300,:300] = eye(300), A[b,300:,300:] = 0
  - H_sub = concat(H_m, H_p) -> pure DMA.

So the kernel is memory-bound: stream H_m/H_p through to H_sub, and write
A_sub from SBUF-resident templates patched per batch with the s-dependent
parts.  A_mm (184 MB) is never used and is not transferred.

Sharding: pure data parallelism, batch 512 -> 64 per core across 8 cores.
"""

import numpy as np

import concourse.bass as bass
import concourse.bacc as bacc
import concourse.mybir as mybir
from concourse.tile import TileContext
from concourse.vector_clock import ScopedClock
from concourse.bass_utils import run_bass_kernel_spmd

F32 = mybir.dt.float32

N_CORES = 8
B_TOTAL = 512
BL = B_TOTAL // N_CORES        # 64 batches per core
M = 300                        # mhc nodes
P = 24                         # peptide nodes
D = 512                        # hidden
SIZE = 324                     # 300 + 24
TOPK = 20
AD = 64                        # attention dim
GB = 8                         # batches per A_sub slab group
NG = BL // GB                  # groups per core
RING = 2                       # slab ring depth
HM_CHUNK = 8                   # batches per H_m dram->dram DMA

INDICES = [7, 9, 24, 45, 59, 62, 63, 66, 67, 69, 70, 73, 74, 76, 77, 80, 81,
           84, 95, 97, 99, 114, 116, 118, 143, 147, 150, 152, 156, 158, 159,
           163, 167, 171]


class _SplitDrainTileContext(TileContext):
    """TileContext whose tail drain carries at most one sem wait.

    The pinned walrus rejects a TPB_CTRL (Drain) instruction with more than
    one sync-wait ("Too many sync wait commands", CoreV3GenImpl.cpp:104), and
    the stock tail drain waits on every outstanding sem lane.  Split it into
    a chain of single-wait drains on SP; semantically identical.
    """

    def _drain_and_barrier(self, tick_clock, wait_clock):
        drain_inst = self.nc.sync.drain()
        wait_clock.add_sem_waits(
            drain_inst.ins, ScopedClock({None: tick_clock.global_clock})
        )
        si = drain_inst.ins.sync_info
        if si is not None and si.on_wait and len(si.on_wait) > 1:
            waits = list(si.on_wait)
            drain_inst.ins.sync_info = mybir.SyncInfo(
                on_wait=[waits[0]], on_update=list(si.on_update or [])
            )
            for w in waits[1:]:
                d2 = self.nc.sync.drain()
                d2.ins.sync_info = mybir.SyncInfo(on_wait=[w], on_update=[])
        self.nc.all_engine_barrier()
        assert self.sems is not None
        popped = self.nc._tile_sem_poison_stack.pop()
        assert popped is self._sem_poison
        self.nc.clear_and_free_semaphores(list(self.sems.allocated().values()))
        self.nc.all_engine_barrier()


def build_kernel() -> bass.Bass:
    nc = bacc.Bacc("TRN2")

    H_m = nc.dram_tensor("H_m", [BL, M, D], F32, kind="ExternalInput")
    H_p = nc.dram_tensor("H_p", [BL, P, D], F32, kind="ExternalInput")
    Wk = nc.dram_tensor("Wk", [AD, D], F32, kind="ExternalInput")
    av = nc.dram_tensor("att_vec", [2 * AD], F32, kind="ExternalInput")
    A_out = nc.dram_tensor("A_sub", [BL, SIZE, SIZE], F32, kind="ExternalOutput")
    H_out = nc.dram_tensor("H_sub", [BL, SIZE, D], F32, kind="ExternalOutput")

    with _SplitDrainTileContext(nc) as tc:
        with (
            tc.tile_pool(name="big", bufs=1) as big,
            tc.tile_pool(name="small", bufs=1) as small,
            tc.tile_pool(name="pw", bufs=1, space="PSUM") as pw,
            tc.tile_pool(name="pstrip", bufs=3, space="PSUM") as pstrip,
        ):
            # ---------- loads ----------
            hp_sb = big.tile([BL, P * D], F32, name="hp_sb")        # 48 KB/part
            nc.sync.dma_start(out=hp_sb[:], in_=H_p[:])

            wk_sb = small.tile([AD, D], F32, name="wk_sb")
            nc.sync.dma_start(out=wk_sb[:], in_=Wk[:])
            a2_col = small.tile([AD, 1], F32, name="a2_col")
            nc.sync.dma_start(out=a2_col[:], in_=av[AD:2 * AD])

            # ---------- constants ----------
            ones128 = small.tile([128, M], F32, name="ones128")
            nc.vector.memset(ones128[:], 1.0)
            t_row = small.tile([1, P], F32, name="t_row")
            nc.vector.memset(t_row[:, 0:TOPK], 1.0)
            nc.vector.memset(t_row[:, TOPK:P], 0.0)

            # u as a single row [1, 300]; strided views u[4q+j] are the
            # lhsT operands of the strip outer-product matmuls
            u4row = small.tile([1, M], F32, name="u4row")
            nc.vector.memset(u4row[:], 0.0)
            for idx in INDICES:
                nc.vector.memset(u4row[:, idx:idx + 1], 1.0)

            # U324/T324: bottom-row constants (24 partitions)
            U324 = small.tile([P, SIZE], F32, name="U324")
            nc.vector.memset(U324[:], 0.0)
            for idx in INDICES:
                nc.vector.memset(U324[:, idx:idx + 1], 1.0)
            T324 = small.tile([P, SIZE], F32, name="T324")
            nc.vector.memset(T324[:], 0.0)
            nc.vector.memset(T324[0:TOPK, 0:M], 1.0)

            # eye masters + identity (GpSimd affine_select is slow: build each
            # pattern ONCE, replicate with broadcast-read copies on ScalarE)
            I64 = small.tile([BL, BL], F32, name="I64")
            nc.gpsimd.affine_select(
                I64[:], ones128[0:BL, 0:BL], pattern=[[1, BL]],
                compare_op=mybir.AluOpType.is_equal, fill=0.0,
                base=0, channel_multiplier=-1,
            )
            # eye master in 4-rows-per-partition layout: partition q holds
            # rows 4q..4q+3; eye4[q, (r, c)] = 1 iff c == 4q + r
            QP = M // 4                                   # 75 partitions
            RW = 4 * SIZE                                 # 1296 elems/slot
            ones4 = small.tile([QP, RW], F32, name="ones4")
            nc.vector.memset(ones4[:], 1.0)
            eye4 = small.tile([QP, RW], F32, name="eye4")
            nc.gpsimd.affine_select(
                eye4[:].rearrange("q (r c) -> q r c", c=SIZE),
                ones4[:].rearrange("q (r c) -> q r c", c=SIZE),
                pattern=[[-1, 4], [1, SIZE]],
                compare_op=mybir.AluOpType.is_equal, fill=0.0,
                base=0, channel_multiplier=-4)

            # All bulk transfers go on the single SWDGE queue (q0), hand-
            # interleaved: H_m chunks first (independent), then alternating
            # with A_sub group writes as they are produced.  One FIFO =
            # engines never starve and no cross-queue priority effects.
            for c in range(8):
                g0 = c * HM_CHUNK
                nc.gpsimd.dma_start(out=H_out[g0:g0 + HM_CHUNK, 0:M, :],
                                    in_=H_m[g0:g0 + HM_CHUNK])

            # ---------- w = Wk.T @ a2; replications via PE outer products ---
            # (keeps POOL off the compute critical path entirely)
            w_ps = pw.tile([1, D], F32, tag="wps")
            nc.tensor.matmul(w_ps[:], lhsT=a2_col[:], rhs=wk_sb[:])
            w_row = small.tile([1, D], F32, name="w_row")
            nc.vector.tensor_copy(out=w_row[:], in_=w_ps[:])
            # w replicated to 64 partitions: ones(64) ⊗ w  (PSUM-resident)
            wrep_ps = pw.tile([BL, D], F32, tag="wrep")
            nc.tensor.matmul(wrep_ps[:], lhsT=ones128[0:1, 0:BL], rhs=w_row[:])
            # t replicated to 64 partitions (for d01) and 128 (strip template)
            trep_ps = pw.tile([BL, P], F32, tag="trep")
            nc.tensor.matmul(trep_ps[:], lhsT=ones128[0:1, 0:BL], rhs=t_row[:])
            # t tiled x4 then replicated to 75 partitions (strip template)
            t4_row = small.tile([1, 4 * P], F32, name="t4_row")
            nc.vector.tensor_copy(
                out=t4_row[:].rearrange("a (r p) -> a r p", p=P),
                in_=t_row[:].unsqueeze(1).to_broadcast([1, 4, P]))
            tstrip_ps = pw.tile([QP, 4 * P], F32, tag="tstrip")
            nc.tensor.matmul(tstrip_ps[:], lhsT=ones128[0:1, 0:QP], rhs=t4_row[:])
            T4_strip = small.tile([QP, 4 * P], F32, name="T4_strip")
            nc.vector.tensor_copy(out=T4_strip[:], in_=tstrip_ps[:])

            # ---------- A_sub slab templates (4 rows per partition) -------
            slabs4 = [big.tile([QP, GB * RW], F32, name=f"slab4_{r}")
                      for r in range(RING)]
            slabs2a = [big.tile([P, GB * SIZE], F32, name=f"slab2a_{r}")
                       for r in range(RING)]

            for r in range(RING):
                # replicate the eye master into all GB slots in one
                # broadcast-read copy per slab (on ScalarE; ACT is idle early)
                nc.scalar.copy(
                    out=slabs4[r][:].rearrange("q (g x) -> q g x", x=RW),
                    in_=eye4[:].unsqueeze(1).to_broadcast([QP, GB, RW]))

            # H_sub peptide rows back out of SBUF (ACT stream, before A DMAs)
            nc.scalar.dma_start(out=H_out[:, M:SIZE, :], in_=hp_sb[:])

            # ---------- sp[b,p] = sum_d H_p[b,p,d] * w[d] ----------
            sp = small.tile([BL, P], F32, name="sp")
            dummy = small.tile([BL, D], F32, name="dummy")
            for p in range(P):
                nc.vector.scalar_tensor_tensor(
                    out=dummy[:], in0=hp_sb[:, p * D:(p + 1) * D], scalar=1.0,
                    in1=wrep_ps[:], op0=mybir.AluOpType.mult,
                    op1=mybir.AluOpType.mult, accum_out=sp[:, p:p + 1],
                )

            # ---------- s01 = top-20 mask of sp per batch ----------
            cmp = small.tile([BL, P * P], F32, name="cmp")
            cmp3 = cmp[:].rearrange("b (p q) -> b p q", q=P)
            sp_q = sp[:].unsqueeze(1).to_broadcast([BL, P, P])   # (b,p,q)->sp[b,q]
            sp_p = sp[:].unsqueeze(2).to_broadcast([BL, P, P])   # (b,p,q)->sp[b,p]
            nc.vector.tensor_tensor(out=cmp3, in0=sp_q, in1=sp_p,
                                    op=mybir.AluOpType.is_lt)
            cnt = small.tile([BL, P], F32, name="cnt")
            nc.vector.tensor_reduce(out=cnt[:], in_=cmp3, axis=mybir.AxisListType.X,
                                    op=mybir.AluOpType.add)
            s01 = small.tile([BL, P], F32, name="s01")
            nc.vector.tensor_scalar(out=s01[:], in0=cnt[:], scalar1=float(P - TOPK) - 0.5,
                                    scalar2=None, op0=mybir.AluOpType.is_gt)
            # d01 = s01 - t  (t replicated via PE, PSUM-resident)
            d01 = small.tile([BL, P], F32, name="d01")
            nc.vector.tensor_sub(out=d01[:], in0=s01[:], in1=trep_ps[:])

            # d01T[p, b] = d01[b, p] via PE transpose
            dT_ps = pw.tile([P, BL], F32, tag="dTps")
            nc.tensor.matmul(dT_ps[:], lhsT=d01[:], rhs=I64[:])
            d01T = small.tile([P, BL], F32, name="d01T")
            nc.vector.tensor_copy(out=d01T[:], in_=dT_ps[:])

            # dRow_all[0, b*24+p] = d01[b, p]: partition->free flatten via a
            # small SBUF->SBUF DMA on ACT (compute-gated; ACT reaches it at
            # about the same time d01 lands, so nothing stalls)
            dRow_all = small.tile([1, BL * P], F32, name="dRow_all")
            nc.sync.dma_start(
                out=dRow_all[:].rearrange("a (g p) -> a g p", p=P),
                in_=d01[:],
            )

            # ---------- per-batch A_sub assembly + streaming ----------
            for b in range(BL):
                g, slot = divmod(b, GB)
                r = g % RING
                c0 = slot * SIZE
                d_row = dRow_all[:, b * P:(b + 1) * P]       # [1, 24]

                # strips for rows 0:300 in 4-row layout: block j gets
                # u[4q+j] ⊗ d_b (4 outer products into one PSUM bank), then
                # + t template in one DVE add straight into the slab
                sps = pstrip.tile([QP, 4 * P], F32, tag="ps")
                for j in range(4):
                    u_j = u4row[:].rearrange("a (q j) -> a q j", j=4)[:, :, j:j + 1]
                    nc.tensor.matmul(sps[:, j * P:(j + 1) * P],
                                     lhsT=u_j, rhs=d_row)
                strip_dst = slabs4[r][:].rearrange(
                    "q (g r c) -> q g r c", r=4, c=SIZE)[:, slot, :, M:SIZE]
                nc.vector.tensor_add(out=strip_dst, in0=sps[:], in1=T4_strip[:])

                # bottom rows: U324*d_b + T324, covers all 324 cols
                nc.vector.scalar_tensor_tensor(
                    out=slabs2a[r][:, c0:c0 + SIZE], in0=U324[:],
                    scalar=d01T[:, b:b + 1], in1=T324[:],
                    op0=mybir.AluOpType.mult, op1=mybir.AluOpType.add,
                )

                if slot == GB - 1:
                    g0 = g * GB
                    src4 = slabs4[r][:].rearrange("q (g x) -> q g x", x=RW)
                    dst4 = A_out[g0:g0 + GB, 0:M, :].rearrange(
                        "b (q r) c -> q b r c", r=4)
                    nc.scalar.dma_start(out=dst4, in_=src4)
                    src2a = slabs2a[r][:].rearrange("p (g c) -> p g c", c=SIZE)
                    nc.scalar.dma_start(
                        out=A_out[g0:g0 + GB, M:SIZE, :].transpose([1, 0, 2]),
                        in_=src2a)

            # H_p rows of H_sub, last on the bulk queue
            nc.gpsimd.dma_start(out=H_out[:, M:SIZE, :], in_=hp_sb[:])

    nc.compile()
    return nc


_NC_CACHE: list = []


def _get_nc() -> bass.Bass:
    if not _NC_CACHE:
        _NC_CACHE.append(build_kernel())
    return _NC_CACHE[0]


def run_sharded(inputs: dict, **spmd_kwargs):
    """Shard full inputs over 8 cores, run, and return (results, perf)."""
    H_m = np.ascontiguousarray(np.asarray(inputs["H_m"], dtype=np.float32))
    H_p = np.ascontiguousarray(np.asarray(inputs["H_p"], dtype=np.float32))
    Wk = np.ascontiguousarray(np.asarray(inputs["Wk"], dtype=np.float32))
    av = np.ascontiguousarray(np.asarray(inputs["att_vec"], dtype=np.float32))
    assert H_m.shape == (B_TOTAL, M, D) and H_p.shape == (B_TOTAL, P, D)

    nc = _get_nc()
    in_maps = []
    for c in range(N_CORES):
        sl = slice(c * BL, (c + 1) * BL)
        in_maps.append({
            "H_m": H_m[sl],
            "H_p": H_p[sl],
            "Wk": Wk,
            "att_vec": av,
        })
    res = run_bass_kernel_spmd(nc, in_maps, core_ids=list(range(N_CORES)),
                               **spmd_kwargs)
    A = np.concatenate([res.results[c]["A_sub"] for c in range(N_CORES)], axis=0)
    H = np.concatenate([res.results[c]["H_sub"] for c in range(N_CORES)], axis=0)
    return (A, H), res


def kernel(H_m, H_p, A_mm, Wq, bq, Wk, bk, att_vec):
    """Full-input entry point: returns (A_sub, H_sub) like the reference."""
    out, _ = run_sharded({"H_m": H_m, "H_p": H_p, "Wk": Wk, "att_vec": att_vec})
    return out


# revision 26
# speedup vs baseline: 1.2542x; 1.1896x over previous
"""Trainium2 Bass kernel for nn_AttentionSubgraphExtractor (topk_masking).

Math notes (derived from the reference):
  - Q/Wq/bq cancel: softmax over the peptide axis removes any per-(b,m)
    constant, so attention depends only on sp[b,p] = H_p[b,p,:] @ (Wk.T @ a2)
    (bk shifts all p equally -> no effect on ordering).
  - softmax is monotonic -> top-k set of att == top-k set of sp.
  - mask is constant per row m: 0 for m in INDICES else -1e9.  att in (0,1)
    added to -1e9 is exactly -1e9 in f32, so masked rows are all-tied and
    jax.lax.top_k returns indices [0..19].
  - The scatter only cares about the SET of top-20 indices.  With s[p] =
    1 if p in top-20 of sp[b], t[p] = 1 if p < 20, u[m] = 1 if m in INDICES:
      A[b, m, 300+p]  = u[m]*s[p] + (1-u[m])*t[p]      (m < 300)
      A[b, 300+p, c]  = u[c]*s[p] + (1-u[c])*t[p]      (c < 300), 0 for c>=300
      A[b, :300,:300] = eye(300), A[b,300:,300:] = 0
  - H_sub = concat(H_m, H_p) -> pure DMA.

So the kernel is memory-bound: stream H_m/H_p through to H_sub, and write
A_sub from SBUF-resident templates patched per batch with the s-dependent
parts.  A_mm (184 MB) is never used and is not transferred.

Sharding: pure data parallelism, batch 512 -> 64 per core across 8 cores.
"""

import numpy as np

import concourse.bass as bass
import concourse.bacc as bacc
import concourse.mybir as mybir
from concourse.tile import TileContext
from concourse.vector_clock import ScopedClock
from concourse.bass_utils import run_bass_kernel_spmd

F32 = mybir.dt.float32

N_CORES = 8
B_TOTAL = 512
BL = B_TOTAL // N_CORES        # 64 batches per core
M = 300                        # mhc nodes
P = 24                         # peptide nodes
D = 512                        # hidden
SIZE = 324                     # 300 + 24
TOPK = 20
AD = 64                        # attention dim
GB = 8                         # batches per A_sub slab group
NG = BL // GB                  # groups per core
RING = 3                       # slab ring depth
HM_CHUNK = 8                   # batches per H_m dram->dram DMA

INDICES = [7, 9, 24, 45, 59, 62, 63, 66, 67, 69, 70, 73, 74, 76, 77, 80, 81,
           84, 95, 97, 99, 114, 116, 118, 143, 147, 150, 152, 156, 158, 159,
           163, 167, 171]


class _SplitDrainTileContext(TileContext):
    """TileContext whose tail drain carries at most one sem wait.

    The pinned walrus rejects a TPB_CTRL (Drain) instruction with more than
    one sync-wait ("Too many sync wait commands", CoreV3GenImpl.cpp:104), and
    the stock tail drain waits on every outstanding sem lane.  Split it into
    a chain of single-wait drains on SP; semantically identical.
    """

    def _drain_and_barrier(self, tick_clock, wait_clock):
        drain_inst = self.nc.sync.drain()
        wait_clock.add_sem_waits(
            drain_inst.ins, ScopedClock({None: tick_clock.global_clock})
        )
        si = drain_inst.ins.sync_info
        if si is not None and si.on_wait and len(si.on_wait) > 1:
            waits = list(si.on_wait)
            drain_inst.ins.sync_info = mybir.SyncInfo(
                on_wait=[waits[0]], on_update=list(si.on_update or [])
            )
            for w in waits[1:]:
                d2 = self.nc.sync.drain()
                d2.ins.sync_info = mybir.SyncInfo(on_wait=[w], on_update=[])
        self.nc.all_engine_barrier()
        assert self.sems is not None
        popped = self.nc._tile_sem_poison_stack.pop()
        assert popped is self._sem_poison
        self.nc.clear_and_free_semaphores(list(self.sems.allocated().values()))
        self.nc.all_engine_barrier()


def build_kernel() -> bass.Bass:
    nc = bacc.Bacc("TRN2")

    H_m = nc.dram_tensor("H_m", [BL, M, D], F32, kind="ExternalInput")
    H_p = nc.dram_tensor("H_p", [BL, P, D], F32, kind="ExternalInput")
    Wk = nc.dram_tensor("Wk", [AD, D], F32, kind="ExternalInput")
    av = nc.dram_tensor("att_vec", [2 * AD], F32, kind="ExternalInput")
    A_out = nc.dram_tensor("A_sub", [BL, SIZE, SIZE], F32, kind="ExternalOutput")
    H_out = nc.dram_tensor("H_sub", [BL, SIZE, D], F32, kind="ExternalOutput")

    with _SplitDrainTileContext(nc) as tc:
        with (
            tc.tile_pool(name="big", bufs=1) as big,
            tc.tile_pool(name="small", bufs=1) as small,
            tc.tile_pool(name="pw", bufs=1, space="PSUM") as pw,
            tc.tile_pool(name="pstrip", bufs=3, space="PSUM") as pstrip,
        ):
            # ---------- loads ----------
            hp_sb = big.tile([BL, P * D], F32, name="hp_sb")        # 48 KB/part
            nc.sync.dma_start(out=hp_sb[:], in_=H_p[:])

            wk_sb = small.tile([AD, D], F32, name="wk_sb")
            nc.sync.dma_start(out=wk_sb[:], in_=Wk[:])
            a2_col = small.tile([AD, 1], F32, name="a2_col")
            nc.sync.dma_start(out=a2_col[:], in_=av[AD:2 * AD])

            # ---------- constants ----------
            ones128 = small.tile([128, M], F32, name="ones128")
            nc.vector.memset(ones128[:], 1.0)
            t_row = small.tile([1, P], F32, name="t_row")
            nc.vector.memset(t_row[:, 0:TOPK], 1.0)
            nc.vector.memset(t_row[:, TOPK:P], 0.0)

            # u-rows: lhsT for the strip outer-product matmuls (K=1)
            U0row = small.tile([1, 128], F32, name="U0row")
            U1row = small.tile([1, 128], F32, name="U1row")
            nc.vector.memset(U0row[:], 0.0)
            nc.vector.memset(U1row[:], 0.0)
            for idx in INDICES:
                if idx < 128:
                    nc.vector.memset(U0row[:, idx:idx + 1], 1.0)
                else:
                    nc.vector.memset(U1row[:, idx - 128:idx - 127], 1.0)

            # U324/T324: bottom-row constants (24 partitions)
            U324 = small.tile([P, SIZE], F32, name="U324")
            nc.vector.memset(U324[:], 0.0)
            for idx in INDICES:
                nc.vector.memset(U324[:, idx:idx + 1], 1.0)
            T324 = small.tile([P, SIZE], F32, name="T324")
            nc.vector.memset(T324[:], 0.0)
            nc.vector.memset(T324[0:TOPK, 0:M], 1.0)

            # eye masters + identity (GpSimd affine_select is slow: build each
            # pattern ONCE, replicate with broadcast-read copies on ScalarE)
            I64 = small.tile([BL, BL], F32, name="I64")
            nc.gpsimd.affine_select(
                I64[:], ones128[0:BL, 0:BL], pattern=[[1, BL]],
                compare_op=mybir.AluOpType.is_equal, fill=0.0,
                base=0, channel_multiplier=-1,
            )
            eye0 = small.tile([128, M], F32, name="eye0")
            eye1 = small.tile([128, M], F32, name="eye1")
            eye2 = small.tile([M - 256, M], F32, name="eye2")
            nc.gpsimd.affine_select(
                eye0[:], ones128[:, 0:M], pattern=[[1, M]],
                compare_op=mybir.AluOpType.is_equal, fill=0.0,
                base=0, channel_multiplier=-1)
            nc.gpsimd.affine_select(
                eye1[:], ones128[:, 0:M], pattern=[[1, M]],
                compare_op=mybir.AluOpType.is_equal, fill=0.0,
                base=-128, channel_multiplier=-1)
            nc.gpsimd.affine_select(
                eye2[:], ones128[0:M - 256, 0:M], pattern=[[1, M]],
                compare_op=mybir.AluOpType.is_equal, fill=0.0,
                base=-256, channel_multiplier=-1)

            # H_m dram->dram chunks via SWDGE (queue 0): start immediately,
            # run at full rate, and their DMASW completion-sem lanes are
            # disjoint from the HWDGE lanes the A_sub pipeline uses, so the
            # bulk stream never inherits waits on compute-gated A-writes.
            for g0 in range(0, BL, HM_CHUNK):
                nc.gpsimd.dma_start(out=H_out[g0:g0 + HM_CHUNK, 0:M, :],
                                    in_=H_m[g0:g0 + HM_CHUNK])

            # ---------- w = Wk.T @ a2; replications via PE outer products ---
            # (keeps POOL off the compute critical path entirely)
            w_ps = pw.tile([1, D], F32, tag="wps")
            nc.tensor.matmul(w_ps[:], lhsT=a2_col[:], rhs=wk_sb[:])
            w_row = small.tile([1, D], F32, name="w_row")
            nc.vector.tensor_copy(out=w_row[:], in_=w_ps[:])
            # w replicated to 64 partitions: ones(64) ⊗ w  (PSUM-resident)
            wrep_ps = pw.tile([BL, D], F32, tag="wrep")
            nc.tensor.matmul(wrep_ps[:], lhsT=ones128[0:1, 0:BL], rhs=w_row[:])
            # t replicated to 64 partitions (for d01) and 128 (strip template)
            trep_ps = pw.tile([BL, P], F32, tag="trep")
            nc.tensor.matmul(trep_ps[:], lhsT=ones128[0:1, 0:BL], rhs=t_row[:])
            tstrip_ps = pw.tile([128, P], F32, tag="tstrip")
            nc.tensor.matmul(tstrip_ps[:], lhsT=ones128[0:1, 0:128], rhs=t_row[:])
            T_strip = small.tile([128, P], F32, name="T_strip")
            nc.vector.tensor_copy(out=T_strip[:], in_=tstrip_ps[:])

            # ---------- A_sub slab templates ----------
            slabs0 = [big.tile([128, GB * SIZE], F32, name=f"slab0_{r}")
                      for r in range(RING)]
            slabs1 = [big.tile([128, GB * SIZE], F32, name=f"slab1_{r}")
                      for r in range(RING)]
            slabs2a = [big.tile([P, GB * SIZE], F32, name=f"slab2a_{r}")
                       for r in range(RING)]
            slab2b = big.tile([M - 256, GB * SIZE], F32, name="slab2b")

            for r in range(RING):
                # replicate the eye masters into all GB slots in one
                # broadcast-read copy per slab (on ScalarE; ACT is idle early)
                nc.scalar.copy(
                    out=slabs0[r][:].rearrange("p (g c) -> p g c", c=SIZE)[:, :, 0:M],
                    in_=eye0[:].unsqueeze(1).to_broadcast([128, GB, M]))
                nc.scalar.copy(
                    out=slabs1[r][:].rearrange("p (g c) -> p g c", c=SIZE)[:, :, 0:M],
                    in_=eye1[:].unsqueeze(1).to_broadcast([128, GB, M]))
            nc.scalar.copy(
                out=slab2b[:].rearrange("p (g c) -> p g c", c=SIZE)[:, :, 0:M],
                in_=eye2[:].unsqueeze(1).to_broadcast([M - 256, GB, M]))
            # rows 256..299 constant t strip (u[m]=0 there): two strided memsets
            s2b = slab2b[:].rearrange("p (g c) -> p g c", c=SIZE)
            nc.vector.memset(s2b[:, :, M:M + TOPK], 1.0)
            nc.vector.memset(s2b[:, :, M + TOPK:SIZE], 0.0)

            # H_sub peptide rows back out of SBUF (ACT stream, before A DMAs)
            nc.scalar.dma_start(out=H_out[:, M:SIZE, :], in_=hp_sb[:])

            # ---------- sp[b,p] = sum_d H_p[b,p,d] * w[d] ----------
            sp = small.tile([BL, P], F32, name="sp")
            dummy = small.tile([BL, D], F32, name="dummy")
            for p in range(P):
                nc.vector.scalar_tensor_tensor(
                    out=dummy[:], in0=hp_sb[:, p * D:(p + 1) * D], scalar=1.0,
                    in1=wrep_ps[:], op0=mybir.AluOpType.mult,
                    op1=mybir.AluOpType.mult, accum_out=sp[:, p:p + 1],
                )

            # ---------- s01 = top-20 mask of sp per batch ----------
            cmp = small.tile([BL, P * P], F32, name="cmp")
            cmp3 = cmp[:].rearrange("b (p q) -> b p q", q=P)
            sp_q = sp[:].unsqueeze(1).to_broadcast([BL, P, P])   # (b,p,q)->sp[b,q]
            sp_p = sp[:].unsqueeze(2).to_broadcast([BL, P, P])   # (b,p,q)->sp[b,p]
            nc.vector.tensor_tensor(out=cmp3, in0=sp_q, in1=sp_p,
                                    op=mybir.AluOpType.is_lt)
            cnt = small.tile([BL, P], F32, name="cnt")
            nc.vector.tensor_reduce(out=cnt[:], in_=cmp3, axis=mybir.AxisListType.X,
                                    op=mybir.AluOpType.add)
            s01 = small.tile([BL, P], F32, name="s01")
            nc.vector.tensor_scalar(out=s01[:], in0=cnt[:], scalar1=float(P - TOPK) - 0.5,
                                    scalar2=None, op0=mybir.AluOpType.is_gt)
            # d01 = s01 - t  (t replicated via PE, PSUM-resident)
            d01 = small.tile([BL, P], F32, name="d01")
            nc.vector.tensor_sub(out=d01[:], in0=s01[:], in1=trep_ps[:])

            # d01T[p, b] = d01[b, p] via PE transpose
            dT_ps = pw.tile([P, BL], F32, tag="dTps")
            nc.tensor.matmul(dT_ps[:], lhsT=d01[:], rhs=I64[:])
            d01T = small.tile([P, BL], F32, name="d01T")
            nc.vector.tensor_copy(out=d01T[:], in_=dT_ps[:])

            # dRow_all[0, b*24+p] = d01[b, p]: partition->free flatten via a
            # small SBUF->SBUF DMA on ACT (compute-gated; ACT reaches it at
            # about the same time d01 lands, so nothing stalls)
            dRow_all = small.tile([1, BL * P], F32, name="dRow_all")
            nc.scalar.dma_start(
                out=dRow_all[:].rearrange("a (g p) -> a g p", p=P),
                in_=d01[:],
            )

            # slab2b is fully constant: its 8 group DMAs can stream any time
            for g in range(NG):
                g0 = g * GB
                nc.scalar.dma_start(
                    out=A_out[g0:g0 + GB, 256:M, :].transpose([1, 0, 2]),
                    in_=slab2b[:].rearrange("p (g c) -> p g c", c=SIZE))

            # ---------- per-batch A_sub assembly + streaming ----------
            for b in range(BL):
                g, slot = divmod(b, GB)
                r = g % RING
                c0 = slot * SIZE
                d_row = dRow_all[:, b * P:(b + 1) * P]       # [1, 24]

                # strip = u ⊗ d_b  (PE outer product, K=1)  + t template (DVE)
                ps0 = pstrip.tile([128, P], F32, tag="ps")
                nc.tensor.matmul(ps0[:], lhsT=U0row[:], rhs=d_row)
                nc.vector.tensor_add(out=slabs0[r][:, c0 + M:c0 + SIZE],
                                     in0=ps0[:], in1=T_strip[:])

                ps1 = pstrip.tile([128, P], F32, tag="ps")
                nc.tensor.matmul(ps1[:], lhsT=U1row[:], rhs=d_row)
                nc.vector.tensor_add(out=slabs1[r][:, c0 + M:c0 + SIZE],
                                     in0=ps1[:], in1=T_strip[:])

                # bottom rows: U324*d_b + T324, covers all 324 cols
                nc.vector.scalar_tensor_tensor(
                    out=slabs2a[r][:, c0:c0 + SIZE], in0=U324[:],
                    scalar=d01T[:, b:b + 1], in1=T324[:],
                    op0=mybir.AluOpType.mult, op1=mybir.AluOpType.add,
                )

                if slot == GB - 1:
                    g0 = g * GB
                    src0 = slabs0[r][:].rearrange("p (g c) -> p g c", c=SIZE)
                    nc.scalar.dma_start(
                        out=A_out[g0:g0 + GB, 0:128, :].transpose([1, 0, 2]),
                        in_=src0)
                    src1 = slabs1[r][:].rearrange("p (g c) -> p g c", c=SIZE)
                    nc.scalar.dma_start(
                        out=A_out[g0:g0 + GB, 128:256, :].transpose([1, 0, 2]),
                        in_=src1)
                    src2a = slabs2a[r][:].rearrange("p (g c) -> p g c", c=SIZE)
                    nc.scalar.dma_start(
                        out=A_out[g0:g0 + GB, M:SIZE, :].transpose([1, 0, 2]),
                        in_=src2a)

    nc.compile()
    return nc


_NC_CACHE: list = []


def _get_nc() -> bass.Bass:
    if not _NC_CACHE:
        _NC_CACHE.append(build_kernel())
    return _NC_CACHE[0]


def run_sharded(inputs: dict, **spmd_kwargs):
    """Shard full inputs over 8 cores, run, and return (results, perf)."""
    H_m = np.ascontiguousarray(np.asarray(inputs["H_m"], dtype=np.float32))
    H_p = np.ascontiguousarray(np.asarray(inputs["H_p"], dtype=np.float32))
    Wk = np.ascontiguousarray(np.asarray(inputs["Wk"], dtype=np.float32))
    av = np.ascontiguousarray(np.asarray(inputs["att_vec"], dtype=np.float32))
    assert H_m.shape == (B_TOTAL, M, D) and H_p.shape == (B_TOTAL, P, D)

    nc = _get_nc()
    in_maps = []
    for c in range(N_CORES):
        sl = slice(c * BL, (c + 1) * BL)
        in_maps.append({
            "H_m": H_m[sl],
            "H_p": H_p[sl],
            "Wk": Wk,
            "att_vec": av,
        })
    res = run_bass_kernel_spmd(nc, in_maps, core_ids=list(range(N_CORES)),
                               **spmd_kwargs)
    A = np.concatenate([res.results[c]["A_sub"] for c in range(N_CORES)], axis=0)
    H = np.concatenate([res.results[c]["H_sub"] for c in range(N_CORES)], axis=0)
    return (A, H), res


def kernel(H_m, H_p, A_mm, Wq, bq, Wk, bk, att_vec):
    """Full-input entry point: returns (A_sub, H_sub) like the reference."""
    out, _ = run_sharded({"H_m": H_m, "H_p": H_p, "Wk": Wk, "att_vec": att_vec})
    return out
